# revision 6
# baseline (speedup 1.0000x reference)
"""Trainium2 Bass kernel for nn_AllocatingLayer (topk_masking).

Math: out[b,i] = weights[b,i] * [load[b,i] <= 100] where
      load[b,i] = sum_j weights[b,j] * [values[b,j] >= values[b,i]].

Since weights >= 0, load[b,i] is non-increasing in values[b,i], so the mask is
exactly [values[b,i] >= t*_b] for a per-row threshold t*_b.  We find t*_b by
vectorized bisection on F_b(t) = sum_j w[b,j]*[v[b,j] >= t]: maintain
(lo, hi] with F(lo) > 100 >= F(hi); after 26 fp32 halvings the bracket is at
most 1 ulp wide, so no sample value lies strictly inside and the mask
[v >= hi] is bit-exact vs. the reference decision.

Sharding: data-parallel over batch, 4 rows per core, no collectives.
Per-core layout: [128 partitions, 64 free] = 4 rows x 32 segments x 64 elems.
Cross-partition (segment) reduction + per-row broadcast are fused into one
TensorEngine matmul with a constant 0/1 "same-row" matrix G[k,p] = [k//32 == p//32].
"""

import os

import numpy as np

import concourse.bacc as bacc
import concourse.mybir as mybir
from concourse.bass_utils import run_bass_kernel_spmd
from concourse.tile import TileContext

N_CORES = 8
B, K = 32, 2048
RPC = B // N_CORES  # rows per core = 4
SEG = 32  # segments per row
FREE = K // SEG  # 64
P = RPC * SEG  # 128 partitions
N_ITERS = 26
W_RESOURCE = 100.0

_last_exec_ns = None
_last_results = None
_nc_cache = None


def _build_nc():
    nc = bacc.Bacc("TRN2", target_bir_lowering=False)
    v_ext = nc.declare_dram_parameter(
        "values", [RPC, K], mybir.dt.float32, isOutput=False
    )
    w_ext = nc.declare_dram_parameter(
        "weights", [RPC, K], mybir.dt.float32, isOutput=False
    )
    g_ext = nc.declare_dram_parameter(
        "gmat", [128, 128], mybir.dt.float32, isOutput=False
    )
    o_ext = nc.declare_dram_parameter("out", [RPC, K], mybir.dt.float32, isOutput=True)

    v_r = v_ext[:].rearrange("r (s f) -> (r s) f", s=SEG)
    w_r = w_ext[:].rearrange("r (s f) -> (r s) f", s=SEG)
    o_r = o_ext[:].rearrange("r (s f) -> (r s) f", s=SEG)

    with TileContext(nc) as tc:
        with (
            tc.tile_pool(name="persist", bufs=1) as persist,
            tc.tile_pool(name="work", bufs=2) as work,
            tc.tile_pool(name="psum", bufs=2, space="PSUM") as psum,
        ):
            v128 = persist.tile([P, FREE], mybir.dt.float32)
            w128 = persist.tile([P, FREE], mybir.dt.float32)
            g128 = persist.tile([128, 128], mybir.dt.float32)
            nc.sync.dma_start(out=v128, in_=v_r)
            nc.sync.dma_start(out=w128, in_=w_r)
            nc.sync.dma_start(out=g128, in_=g_ext[:])

            lo = persist.tile([P, 1], mybir.dt.float32)
            hi = persist.tile([P, 1], mybir.dt.float32)
            nc.vector.memset(lo, 0.0)
            nc.vector.memset(hi, 1.0)

            # Pre-touch the DMA'd tiles with non-Ptr ops so the DMA semaphore
            # waits land here: TensorScalarPtr ops (AP-scalar tensor_scalar /
            # scalar_tensor_tensor) only have a single sync-wait slot.
            scratch = persist.tile([P, 1], mybir.dt.float32)
            nc.vector.tensor_copy(scratch, v128[:, 0:1])
            nc.vector.tensor_copy(scratch, w128[:, 0:1])

            for _ in range(N_ITERS):
                t = work.tile([P, 1], mybir.dt.float32, tag="t")
                # t = (lo + hi) * 0.5
                nc.vector.tensor_scalar(
                    out=t,
                    in0=lo,
                    scalar1=hi[:, 0:1],
                    scalar2=0.5,
                    op0=mybir.AluOpType.add,
                    op1=mybir.AluOpType.mult,
                )
                # m = (v >= t) * w ; spart[p] = sum_f m  (one fused DVE op)
                m = work.tile([P, FREE], mybir.dt.float32, tag="m")
                spart = work.tile([P, 1], mybir.dt.float32, tag="spart")
                nc.vector.scalar_tensor_tensor(
                    out=m,
                    in0=v128,
                    scalar=t[:, 0:1],
                    in1=w128,
                    op0=mybir.AluOpType.is_ge,
                    op1=mybir.AluOpType.mult,
                    accum_out=spart,
                )
                # F[p] = sum_k G[k,p]*spart[k]  (row sum, broadcast to all 32
                # partitions of the row in one matmul)
                fps = psum.tile([128, 1], mybir.dt.float32, tag="fps")
                nc.tensor.matmul(fps, g128, spart, start=True, stop=True)
                d = work.tile([P, 1], mybir.dt.uint32, tag="d")
                dn = work.tile([P, 1], mybir.dt.uint32, tag="dn")
                nc.vector.tensor_scalar(
                    out=d,
                    in0=fps,
                    scalar1=W_RESOURCE,
                    scalar2=None,
                    op0=mybir.AluOpType.is_le,
                )
                nc.vector.tensor_scalar(
                    out=dn,
                    in0=fps,
                    scalar1=W_RESOURCE,
                    scalar2=None,
                    op0=mybir.AluOpType.is_gt,
                )
                # F <= 100 -> hi = t ; F > 100 -> lo = t
                nc.vector.copy_predicated(hi, d, t)
                nc.vector.copy_predicated(lo, dn, t)

            # out = w * (v >= hi)
            mfin = work.tile([P, FREE], mybir.dt.float32, tag="m")
            nc.vector.tensor_scalar(
                out=mfin,
                in0=v128,
                scalar1=hi[:, 0:1],
                scalar2=None,
                op0=mybir.AluOpType.is_ge,
            )
            outt = work.tile([P, FREE], mybir.dt.float32, tag="outt")
            nc.vector.tensor_mul(outt, mfin, w128)
            nc.sync.dma_start(out=o_r, in_=outt)
    nc.compile()
    return nc


def _gmat():
    k = np.arange(128) // SEG
    return np.ascontiguousarray((k[:, None] == k[None, :]).astype(np.float32))


def kernel(values, weights):
    global _nc_cache, _last_exec_ns, _last_results
    v = np.ascontiguousarray(np.asarray(values, dtype=np.float32))
    w = np.ascontiguousarray(np.asarray(weights, dtype=np.float32))
    assert v.shape == (B, K) and w.shape == (B, K)
    if _nc_cache is None:
        _nc_cache = _build_nc()
    g = _gmat()
    in_maps = [
        {
            "values": np.ascontiguousarray(v[i * RPC : (i + 1) * RPC]),
            "weights": np.ascontiguousarray(w[i * RPC : (i + 1) * RPC]),
            "gmat": g,
        }
        for i in range(N_CORES)
    ]
    trace = bool(os.environ.get("KERNEL_TRACE"))
    res = run_bass_kernel_spmd(
        _nc_cache, in_maps, core_ids=list(range(N_CORES)), trace=trace
    )
    _last_exec_ns = res.exec_time_ns
    _last_results = res
    return np.concatenate([res.results[i]["out"] for i in range(N_CORES)], axis=0)


# revision 7
# speedup vs baseline: 1.0515x; 1.0515x over previous
"""Trainium2 Bass kernel for nn_AllocatingLayer (topk_masking).

Math: out[b,i] = weights[b,i] * [load[b,i] <= 100] where
      load[b,i] = sum_j weights[b,j] * [values[b,j] >= values[b,i]].

Since weights >= 0, load[b,i] is non-increasing in values[b,i], so the mask is
exactly [values[b,i] >= t*_b] for a per-row threshold t*_b.  We find t*_b by
vectorized bisection on F_b(t) = sum_j w[b,j]*[v[b,j] >= t]:

- Fast phase (22 evals): midpoint-state bisection t' = t +- 2^-(i+2).  All
  updates are exact in fp32 (sums of distinct powers of two, mantissa span
  <= 23 bits), one fused update op per iteration.
- Tail phase (3 evals): classic (lo,hi] bisection with copy_predicated
  updates; midpoint rounding makes it converge to a <=1-ulp bracket, so no
  sample value lies strictly inside and the final mask [v >= hi] reproduces
  the reference decision bit-exactly.

Sharding: data-parallel over batch, 4 rows per core, no collectives.
Per-core layout: [128 partitions, 64 free] = 4 rows x 32 segments x 64 elems.
The whole iteration runs on the Vector engine: fused compare-mul-accumulate
(scalar_tensor_tensor) -> 32x32 stream transpose -> free-dim reduce ->
compare -> stream_shuffle broadcast (lane 0 -> all lanes of each 32-block).
"""

import os

import numpy as np

import concourse.bacc as bacc
import concourse.mybir as mybir
from concourse.bass_utils import run_bass_kernel_spmd
from concourse.tile import TileContext

N_CORES = 8
B, K = 32, 2048
RPC = B // N_CORES  # rows per core = 4
SEG = 32  # segments per row
FREE = K // SEG  # 64
P = RPC * SEG  # 128 partitions
N_FAST = 22
N_TAIL = 3
W_RESOURCE = 100.0

_last_exec_ns = None
_last_results = None
_nc_cache = None


def _build_nc():
    nc = bacc.Bacc("TRN2", target_bir_lowering=False)
    v_ext = nc.declare_dram_parameter(
        "values", [RPC, K], mybir.dt.float32, isOutput=False
    )
    w_ext = nc.declare_dram_parameter(
        "weights", [RPC, K], mybir.dt.float32, isOutput=False
    )
    o_ext = nc.declare_dram_parameter("out", [RPC, K], mybir.dt.float32, isOutput=True)

    v_r = v_ext[:].rearrange("r (s f) -> (r s) f", s=SEG)
    w_r = w_ext[:].rearrange("r (s f) -> (r s) f", s=SEG)
    o_r = o_ext[:].rearrange("r (s f) -> (r s) f", s=SEG)

    AL = mybir.AluOpType
    f32 = mybir.dt.float32
    BCAST0 = [0] * 32  # stream_shuffle mask: every lane <- lane 0 of its block

    with TileContext(nc) as tc:
        with (
            tc.tile_pool(name="persist", bufs=1) as persist,
            tc.tile_pool(name="work", bufs=2) as work,
        ):
            v128 = persist.tile([P, FREE], f32)
            w128 = persist.tile([P, FREE], f32)
            nc.sync.dma_start(out=v128, in_=v_r)
            nc.sync.dma_start(out=w128, in_=w_r)

            # spad holds per-(row,segment) partial sums in col 0; cols 1..31
            # must be zero so the transposed reduce sees clean zeros.  Two
            # slots rotate, memset both once.
            spads = [persist.tile([P, SEG], f32, name=f"spad{i}") for i in range(2)]
            for s in spads:
                nc.vector.memset(s, 0.0)

            # Pre-touch the DMA'd tiles with non-Ptr ops so the DMA semaphore
            # waits land here (TensorScalarPtr has one sync-wait slot).
            scratch = persist.tile([P, 1], f32)
            nc.vector.tensor_copy(scratch, v128[:, 0:1])
            nc.vector.tensor_copy(scratch, w128[:, 0:1])

            t = persist.tile([P, 1], f32, name="t0")
            nc.vector.memset(t, 0.5)

            def f_eval(thr_col, spad):
                """F(thr) broadcast-ready: returns Fcol [P,1], valid at
                partitions 32r, after STT -> transpose -> reduce."""
                m = work.tile([P, FREE], f32, tag="m")
                nc.vector.scalar_tensor_tensor(
                    out=m,
                    in0=v128,
                    scalar=thr_col,
                    in1=w128,
                    op0=AL.is_ge,
                    op1=AL.mult,
                    accum_out=spad[:, 0:1],
                )
                tp = work.tile([P, SEG], f32, tag="tp")
                nc.vector.transpose(tp, spad)
                fcol = work.tile([P, 1], f32, tag="fcol")
                nc.vector.reduce_sum(fcol, tp, axis=mybir.AxisListType.X)
                return fcol

            # ---- fast phase: t' = t + (F>100 ? +1 : -1) * 2^-(i+2) ----
            for i in range(N_FAST):
                fcol = f_eval(t[:, 0:1], spads[i % 2])
                dnpm = work.tile([P, 1], f32, tag="dnpm")
                nc.vector.tensor_scalar(
                    out=dnpm,
                    in0=fcol,
                    scalar1=W_RESOURCE,
                    scalar2=0.5,
                    op0=AL.is_gt,
                    op1=AL.subtract,
                )
                dnb = work.tile([P, 1], f32, tag="dnb")
                nc.vector.stream_shuffle(dnb, dnpm, BCAST0)
                t_new = work.tile([P, 1], f32, tag="t")
                nc.vector.scalar_tensor_tensor(
                    out=t_new,
                    in0=dnb,
                    scalar=float(2.0 ** -(i + 1)),
                    in1=t,
                    op0=AL.mult,
                    op1=AL.add,
                )
                t = t_new

            # ---- reconstruct bracket ----
            lo = persist.tile([P, 1], f32, name="lo")
            hi = persist.tile([P, 1], f32, name="hi")
            half = float(2.0 ** -(N_FAST + 1))
            nc.vector.tensor_scalar(
                out=lo, in0=t, scalar1=half, scalar2=None, op0=AL.subtract
            )
            nc.vector.tensor_scalar(
                out=hi, in0=t, scalar1=half, scalar2=None, op0=AL.add
            )

            # ---- tail phase: exact-midpoint bisection ----
            for j in range(N_TAIL):
                tm = work.tile([P, 1], f32, tag="tm")
                nc.vector.tensor_scalar(
                    out=tm,
                    in0=lo,
                    scalar1=hi[:, 0:1],
                    scalar2=0.5,
                    op0=AL.add,
                    op1=AL.mult,
                )
                fcol = f_eval(tm[:, 0:1], spads[j % 2])
                du = work.tile([P, 1], mybir.dt.uint32, tag="du")
                dnu = work.tile([P, 1], mybir.dt.uint32, tag="dnu")
                nc.vector.tensor_scalar(
                    out=du, in0=fcol, scalar1=W_RESOURCE, scalar2=None, op0=AL.is_le
                )
                nc.vector.tensor_scalar(
                    out=dnu, in0=fcol, scalar1=W_RESOURCE, scalar2=None, op0=AL.is_gt
                )
                dub = work.tile([P, 1], mybir.dt.uint32, tag="dub")
                dnub = work.tile([P, 1], mybir.dt.uint32, tag="dnub")
                nc.vector.stream_shuffle(dub, du, BCAST0)
                nc.vector.stream_shuffle(dnub, dnu, BCAST0)
                nc.vector.copy_predicated(hi, dub, tm)
                nc.vector.copy_predicated(lo, dnub, tm)

            # ---- final mask: out = w * (v >= hi) ----
            mfin = work.tile([P, FREE], f32, tag="m")
            nc.vector.tensor_scalar(
                out=mfin,
                in0=v128,
                scalar1=hi[:, 0:1],
                scalar2=None,
                op0=AL.is_ge,
            )
            outt = work.tile([P, FREE], f32, tag="outt")
            nc.vector.tensor_mul(outt, mfin, w128)
            nc.sync.dma_start(out=o_r, in_=outt)
    nc.compile()
    return nc


def kernel(values, weights):
    global _nc_cache, _last_exec_ns, _last_results
    v = np.ascontiguousarray(np.asarray(values, dtype=np.float32))
    w = np.ascontiguousarray(np.asarray(weights, dtype=np.float32))
    assert v.shape == (B, K) and w.shape == (B, K)
    if _nc_cache is None:
        _nc_cache = _build_nc()
    in_maps = [
        {
            "values": np.ascontiguousarray(v[i * RPC : (i + 1) * RPC]),
            "weights": np.ascontiguousarray(w[i * RPC : (i + 1) * RPC]),
        }
        for i in range(N_CORES)
    ]
    trace = bool(os.environ.get("KERNEL_TRACE"))
    res = run_bass_kernel_spmd(
        _nc_cache, in_maps, core_ids=list(range(N_CORES)), trace=trace
    )
    _last_exec_ns = res.exec_time_ns
    _last_results = res
    return np.concatenate([res.results[i]["out"] for i in range(N_CORES)], axis=0)


# revision 14
# speedup vs baseline: 1.0516x; 1.0001x over previous
"""Trainium2 Bass kernel for nn_AllocatingLayer (topk_masking).

Math: out[b,i] = weights[b,i] * [load[b,i] <= 100] where
      load[b,i] = sum_j weights[b,j] * [values[b,j] >= values[b,i]].

Since weights >= 0, load[b,i] is non-increasing in values[b,i], so the mask is
exactly [values[b,i] >= t*_b] for a per-row threshold t*_b.  We find t*_b by
vectorized bisection on F_b(t) = sum_j w[b,j]*[v[b,j] >= t]:

- Fast phase (22 evals): midpoint-state bisection t' = t +- 2^-(i+2).  All
  updates are exact in fp32 (sums of distinct powers of two, mantissa span
  <= 23 bits), one fused update op per iteration.
- Tail phase (3 evals): classic (lo,hi] bisection with copy_predicated
  updates; midpoint rounding makes it converge to a <=1-ulp bracket, so no
  sample value lies strictly inside and the final mask [v >= hi] reproduces
  the reference decision bit-exactly.

Sharding: data-parallel over batch, 4 rows per core, no collectives.
Per-core layout: [128 partitions, 64 free] = 4 rows x 32 segments x 64 elems.
Raw bass (no Tile): the whole iteration is a single in-order Vector-engine
instruction stream — fused compare-mul-accumulate (scalar_tensor_tensor) ->
32x32 stream transpose -> free-dim reduce -> compare -> stream_shuffle
broadcast (lane 0 -> all lanes of each 32-block) -> fused threshold update.
Only two cross-engine syncs exist: DMA-in -> DVE and DVE -> DMA-out.
"""

import os

import numpy as np

import concourse.bacc as bacc
import concourse.mybir as mybir
from concourse.bass_utils import run_bass_kernel_spmd

N_CORES = 8
B, K = 32, 2048
RPC = B // N_CORES  # rows per core = 4
SEG = 32  # segments per row
FREE = K // SEG  # 64
P = RPC * SEG  # 128 partitions
N_FAST = 22
N_TAIL = 3
W_RESOURCE = 100.0

_last_exec_ns = None
_last_results = None
_nc_cache = None


def _build_nc():
    nc = bacc.Bacc("TRN2", target_bir_lowering=False)
    f32 = mybir.dt.float32
    u32 = mybir.dt.uint32
    AL = mybir.AluOpType
    BCAST0 = [0] * 32  # stream_shuffle mask: every lane <- lane 0 of its block

    v_ext = nc.declare_dram_parameter("values", [RPC, K], f32, isOutput=False)
    w_ext = nc.declare_dram_parameter("weights", [RPC, K], f32, isOutput=False)
    o_ext = nc.declare_dram_parameter("out", [RPC, K], f32, isOutput=True)

    v_r = v_ext[:].rearrange("r (s f) -> (r s) f", s=SEG)
    w_r = w_ext[:].rearrange("r (s f) -> (r s) f", s=SEG)
    o_r = o_ext[:].rearrange("r (s f) -> (r s) f", s=SEG)

    with (
        nc.sbuf_tensor("v128", [P, FREE], f32) as v128,
        nc.sbuf_tensor("w128", [P, FREE], f32) as w128,
        nc.sbuf_tensor("mbuf", [P, FREE], f32) as mbuf,
        nc.sbuf_tensor("outt", [P, FREE], f32) as outt,
        nc.sbuf_tensor("spad", [P, SEG], f32) as spad,
        nc.sbuf_tensor("tpbuf", [P, SEG], f32) as tpbuf,
        nc.sbuf_tensor("cols", [P, 8], f32) as cols,
        nc.sbuf_tensor("colsu", [P, 4], u32) as colsu,
        nc.semaphore("dma_sem") as dma_sem,
        nc.semaphore("done_sem") as done_sem,
        nc.semaphore("vsem") as vsem,
        nc.Block() as block,
    ):
        t_a = cols[:, 0:1]
        t_b = cols[:, 1:2]
        fcol = cols[:, 2:3]
        dnpm = cols[:, 3:4]
        dnb = cols[:, 4:5]
        lo = cols[:, 5:6]
        hi = cols[:, 6:7]
        tm = cols[:, 7:8]
        du = colsu[:, 0:1]
        dnu = colsu[:, 1:2]
        dub = colsu[:, 2:3]
        dnub = colsu[:, 3:4]

        @block.sync
        def _(sync):
            sync.dma_start(out=v128[:], in_=v_r).then_inc(dma_sem, 16)
            sync.dma_start(out=w128[:], in_=w_r).then_inc(dma_sem, 16)
            sync.wait_ge(done_sem, 1)
            sync.dma_start(out=o_r, in_=outt[:]).then_inc(dma_sem, 16)

        @block.vector
        def _(vector):
            # The DVE does NOT guarantee that a later instruction observes an
            # earlier instruction's SBUF writes (pipelined issue; and the
            # TensorScalarPtr scalar-pointer is fetched by the sequencer at
            # decode time).  Chain every instruction through vsem: wait for
            # the previous op's completion inc, then inc on our own
            # completion.  This is what Tile's scheduler emits per-op.
            vcnt = [0]

            def chain(inst, inc=True):
                if vcnt[0]:
                    inst._wait_ge(vsem, vcnt[0])
                if inc:
                    vcnt[0] += 1
                    inst.then_inc(vsem, 1)
                return inst

            chain(nc.vector.memset(spad[:], 0.0))
            chain(nc.vector.memset(t_a, 0.5))
            vector.wait_ge(dma_sem, 32)

            def f_eval(thr_col):
                """F(thr): STT -> transpose -> reduce; fcol valid at
                partitions 32r."""
                chain(
                    nc.vector.scalar_tensor_tensor(
                        out=mbuf[:],
                        in0=v128[:],
                        scalar=thr_col,
                        in1=w128[:],
                        op0=AL.is_ge,
                        op1=AL.mult,
                        accum_out=spad[:, 0:1],
                    )
                )
                chain(nc.vector.transpose(tpbuf[:], spad[:]))
                chain(nc.vector.reduce_sum(fcol, tpbuf[:], axis=mybir.AxisListType.X))

            # ---- fast phase: t' = t + (F>100 ? +1 : -1) * 2^-(i+2) ----
            t_cur, t_nxt = t_a, t_b
            for i in range(N_FAST):
                f_eval(t_cur)
                chain(
                    nc.vector.tensor_scalar(
                        out=dnpm,
                        in0=fcol,
                        scalar1=W_RESOURCE,
                        scalar2=0.5,
                        op0=AL.is_gt,
                        op1=AL.subtract,
                    )
                )
                chain(nc.vector.stream_shuffle(dnb, dnpm, BCAST0))
                chain(
                    nc.vector.scalar_tensor_tensor(
                        out=t_nxt,
                        in0=dnb,
                        scalar=float(2.0 ** -(i + 1)),
                        in1=t_cur,
                        op0=AL.mult,
                        op1=AL.add,
                    )
                )
                t_cur, t_nxt = t_nxt, t_cur

            # ---- reconstruct bracket ----
            half = float(2.0 ** -(N_FAST + 1))
            chain(
                nc.vector.tensor_scalar(
                    out=lo, in0=t_cur, scalar1=half, scalar2=None, op0=AL.subtract
                )
            )
            chain(
                nc.vector.tensor_scalar(
                    out=hi, in0=t_cur, scalar1=half, scalar2=None, op0=AL.add
                )
            )

            # ---- tail phase: exact-midpoint bisection ----
            for _ in range(N_TAIL):
                chain(
                    nc.vector.tensor_scalar(
                        out=tm,
                        in0=lo,
                        scalar1=hi,
                        scalar2=0.5,
                        op0=AL.add,
                        op1=AL.mult,
                    )
                )
                f_eval(tm)
                chain(
                    nc.vector.tensor_scalar(
                        out=du, in0=fcol, scalar1=W_RESOURCE, scalar2=None, op0=AL.is_le
                    )
                )
                chain(
                    nc.vector.tensor_scalar(
                        out=dnu, in0=fcol, scalar1=W_RESOURCE, scalar2=None, op0=AL.is_gt
                    )
                )
                chain(nc.vector.stream_shuffle(dub, du, BCAST0))
                chain(nc.vector.stream_shuffle(dnub, dnu, BCAST0))
                chain(nc.vector.copy_predicated(hi, dub, tm))
                chain(nc.vector.copy_predicated(lo, dnub, tm))

            # ---- final mask: out = w * (v >= hi) ----
            chain(
                nc.vector.tensor_scalar(
                    out=mbuf[:],
                    in0=v128[:],
                    scalar1=hi,
                    scalar2=None,
                    op0=AL.is_ge,
                )
            )
            chain(
                nc.vector.tensor_mul(outt[:], mbuf[:], w128[:]), inc=False
            ).then_inc(done_sem, 1)

    nc.compile()
    return nc


def kernel(values, weights):
    global _nc_cache, _last_exec_ns, _last_results
    v = np.ascontiguousarray(np.asarray(values, dtype=np.float32))
    w = np.ascontiguousarray(np.asarray(weights, dtype=np.float32))
    assert v.shape == (B, K) and w.shape == (B, K)
    if _nc_cache is None:
        _nc_cache = _build_nc()
    in_maps = [
        {
            "values": np.ascontiguousarray(v[i * RPC : (i + 1) * RPC]),
            "weights": np.ascontiguousarray(w[i * RPC : (i + 1) * RPC]),
        }
        for i in range(N_CORES)
    ]
    trace = bool(os.environ.get("KERNEL_TRACE"))
    res = run_bass_kernel_spmd(
        _nc_cache, in_maps, core_ids=list(range(N_CORES)), trace=trace
    )
    _last_exec_ns = res.exec_time_ns
    _last_results = res
    return np.concatenate([res.results[i]["out"] for i in range(N_CORES)], axis=0)
